# revision 12
# baseline (speedup 1.0000x reference)
"""DiehlCook spiking network on 8 TRN2 NeuronCores — batch data-parallel.

Numerics are a bitwise replication of the jax-on-neuron reference:
  * fp32 PE matmuls, K chunked in 128s ascending, PSUM accumulation
    (verified bitwise against jnp.dot / the reference einsum).
  * LIF elementwise chain with the reference's exact op order:
      u  = (v - v_rest) * decay            (DVE tensor_scalar)
      v2 = (u + v_rest) + d                (DVE scalar_tensor_tensor)
      s  = v2 >= v_th                      (DVE tensor_scalar is_ge)
      v3 = min(s * v_reset, v2)            (DVE stt — exact where(s,reset,v2))
      d  = I_ff[t] - inh                   (GPSIMD tensor_tensor)
  * s_inh[t] == s_exc[t-1] exactly for these LIF constants, so the
    inhibitory layer is never computed: its spike train is the excitatory
    one delayed a step, and the recurrent drive uses s_exc[t-2].
"""
import sys
sys.path.insert(0, '/opt/trn_rl_repo')

import numpy as np

N_CORES = 8
B, N_IN, T, E = 256, 784, 350, 400
BL = B // N_CORES          # 32 batch rows per core
MT = BL * T                # 11200 rows, m = t*BL + b
V_REST, V_RESET, V_TH = -65.0, -60.0, -52.0
DECAY = float(np.exp(-1.0 / 100.0))

_PROG_CACHE = {}


def _build_program(t_run):
    import concourse.bass as bass
    import concourse.tile as tile
    from concourse import mybir, bacc

    f32 = mybir.dt.float32
    n_pairs = t_run // 2
    n_mchunk = (BL * t_run + 127) // 128

    nc = bacc.Bacc("TRN2", target_bir_lowering=False, debug=False,
                   num_devices=N_CORES)
    spT_d = nc.dram_tensor("spT", [N_IN, MT], f32, kind="ExternalInput").ap()
    w_d = nc.dram_tensor("w", [N_IN, E], f32, kind="ExternalInput").ap()
    wie_d = nc.dram_tensor("wie", [E, E], f32, kind="ExternalInput").ap()
    id_d = nc.dram_tensor("ident", [32, 32], f32, kind="ExternalInput").ap()
    iff_d = nc.dram_tensor("iff", [MT, E], f32).ap()               # scratch
    hist_d = nc.dram_tensor("hist", [MT, E], f32, kind="ExternalOutput").ap()

    KC_FF = [(kc * 128, min(128, N_IN - kc * 128)) for kc in range(7)]
    KC_IE = [(kc * 128, min(128, E - kc * 128)) for kc in range(4)]
    EC = [(c * 128, min(128, E - c * 128)) for c in range(4)]      # e-chunks

    with tile.TileContext(nc) as tc:
        with (
            tc.tile_pool(name="wres", bufs=1) as wres,
            tc.tile_pool(name="spst", bufs=3) as spst,
            tc.tile_pool(name="ffps", bufs=2, space="PSUM") as ffps,
            tc.tile_pool(name="ffout", bufs=3) as ffout,
            tc.tile_pool(name="state", bufs=1) as state,
            tc.tile_pool(name="iffin", bufs=3) as iffin,
            tc.tile_pool(name="inhps", bufs=2, space="PSUM") as inhps,
            tc.tile_pool(name="stps", bufs=2, space="PSUM") as stps,
            tc.tile_pool(name="work", bufs=2) as work,
            tc.tile_pool(name="shist", bufs=4) as shist,
        ):
            # ---------------- resident weights ----------------
            w_t = wres.tile([128, 7 * E], f32)
            for kc, (k0, kw) in enumerate(KC_FF):
                nc.gpsimd.dma_start(w_t[:kw, kc * E:(kc + 1) * E], w_d[k0:k0 + kw, :])
            wie_t = wres.tile([128, 4 * E], f32)
            for kc, (k0, kw) in enumerate(KC_IE):
                nc.gpsimd.dma_start(wie_t[:kw, kc * E:(kc + 1) * E], wie_d[k0:k0 + kw, :])
            ident = wres.tile([32, 32], f32)
            nc.gpsimd.dma_start(ident[:], id_d[:])

            # ---------------- phase 1: feed-forward I_ff ----------------
            for m in range(n_mchunk):
                m0 = m * 128
                mw = min(128, BL * t_run - m0)
                sp_t = spst.tile([128, 7 * 128], f32, tag="sp")
                for kc, (k0, kw) in enumerate(KC_FF):
                    nc.sync.dma_start(sp_t[:kw, kc * 128:kc * 128 + mw],
                                      spT_d[k0:k0 + kw, m0:m0 + mw])
                p = ffps.tile([128, E], f32, tag="ffp")
                for kc, (k0, kw) in enumerate(KC_FF):
                    nc.tensor.matmul(p[:mw, :], sp_t[:kw, kc * 128:kc * 128 + mw],
                                     w_t[:kw, kc * E:(kc + 1) * E],
                                     start=(kc == 0), stop=(kc == 6))
                o = ffout.tile([128, E], f32, tag="ffo")
                nc.vector.tensor_copy(o[:mw, :], p[:mw, :])
                nc.sync.dma_start(iff_d[m0:m0 + mw, :], o[:mw, :])

            # ---------------- phase 2: scan ----------------
            v_t = state.tile([32, E], f32)
            nc.vector.memset(v_t[:], V_REST)
            u_t = state.tile([32, E], f32)
            # sT pair buffers: [128, c*64 + step*32 + b]; parity = pair index % 2
            sTbuf0 = state.tile([128, 4 * 64], f32, tag="sT0")
            sTbuf1 = state.tile([128, 4 * 64], f32, tag="sT1")
            sTbuf = [sTbuf0, sTbuf1]
            nc.vector.memset(sTbuf[0][:], 0.0)
            nc.vector.memset(sTbuf[1][:], 0.0)

            for k in range(n_pairs):
                t0 = 2 * k
                iff0 = iffin.tile([32, E], f32, tag="iff0")
                iff1 = iffin.tile([32, E], f32, tag="iff1")
                nc.sync.dma_start(iff0[:], iff_d[t0 * BL:(t0 + 1) * BL, :])
                nc.sync.dma_start(iff1[:], iff_d[(t0 + 1) * BL:(t0 + 2) * BL, :])

                # inhibition for both steps: psum[(step,b), e]
                p_inh = inhps.tile([64, E], f32, tag="inh")
                src = sTbuf[k % 2]
                for kc, (k0, kw) in enumerate(KC_IE):
                    nc.tensor.matmul(p_inh[:, :], src[:kw, kc * 64:(kc + 1) * 64],
                                     wie_t[:kw, kc * E:(kc + 1) * E],
                                     start=(kc == 0), stop=(kc == 3))
                d0 = work.tile([32, E], f32, tag="d0")
                d1 = work.tile([32, E], f32, tag="d1")
                nc.vector.tensor_tensor(d0[:], iff0[:], p_inh[0:32, :],
                                        op=mybir.AluOpType.subtract)
                nc.vector.tensor_tensor(d1[:], iff1[:], p_inh[32:64, :],
                                        op=mybir.AluOpType.subtract)
                d_steps = (d0, d1)

                for step in range(2):
                    t = t0 + step
                    # u1 = v + 65 (DVE); u3 = decay*u1 + (-65) via ACT fused
                    # scale/bias (matches jit_scan's activation fusion incl.
                    # its double-rounding behavior); v = u3 + d (DVE)
                    nc.vector.tensor_scalar(u_t[:], v_t[:], -V_REST, None,
                                            op0=mybir.AluOpType.add)
                    u3_t = work.tile([32, E], f32, tag="u3")
                    nc.scalar.activation(u3_t[:], u_t[:],
                                         mybir.ActivationFunctionType.Copy,
                                         bias=V_REST, scale=DECAY)
                    nc.vector.tensor_tensor(v_t[:], u3_t[:], d_steps[step][:],
                                            op=mybir.AluOpType.add)
                    # s = v >= th
                    s_t = shist.tile([32, E], f32, tag="s")
                    nc.vector.tensor_scalar(s_t[:], v_t[:], V_TH, None,
                                            op0=mybir.AluOpType.is_ge)
                    # v = min(s * reset, v)
                    nc.vector.scalar_tensor_tensor(v_t[:], s_t[:], V_RESET, v_t[:],
                                                   op0=mybir.AluOpType.mult,
                                                   op1=mybir.AluOpType.min)
                    # history out
                    nc.sync.dma_start(hist_d[t * BL:(t + 1) * BL, :], s_t[:])
                    # transpose s into the pair buffer consumed at pair k' = t//2 + 1
                    p_sT = stps.tile([128, 128], f32, tag="sT")
                    for c, (e0, ew) in enumerate(EC):
                        nc.tensor.transpose(p_sT[:ew, c * 32:(c + 1) * 32],
                                            s_t[:, e0:e0 + ew], ident[:])
                    dst = sTbuf[(k + 1) % 2]
                    for c, (e0, ew) in enumerate(EC):
                        nc.scalar.copy(
                            dst[:ew, c * 64 + step * 32: c * 64 + step * 32 + 32],
                            p_sT[:ew, c * 32:(c + 1) * 32])

    nc.finalize()
    return nc


def _get_program(t_run):
    if t_run not in _PROG_CACHE:
        _PROG_CACHE[t_run] = _build_program(t_run)
    return _PROG_CACHE[t_run]


LAST_RESULTS = None


def kernel(input_spikes, w_input_exc, w_inh_exc, inh_exc_mask, _t_run=T,
           _trace=False):
    global LAST_RESULTS
    from concourse.bass_utils import run_bass_kernel_spmd

    spikes = np.asarray(input_spikes, dtype=np.float32)
    w = np.maximum(np.asarray(w_input_exc, dtype=np.float32), 0.0)
    wie = (np.maximum(np.asarray(w_inh_exc, dtype=np.float32), 0.0)
           * np.asarray(inh_exc_mask, dtype=np.float32))
    ident = np.eye(32, dtype=np.float32)

    nc = _get_program(_t_run)
    in_maps = []
    for c in range(N_CORES):
        sl = spikes[c * BL:(c + 1) * BL]                  # (BL, N_IN, T)
        spT = np.ascontiguousarray(
            np.transpose(sl, (1, 2, 0)).reshape(N_IN, MT))  # [n, t*BL+b]
        in_maps.append({"spT": spT, "w": w, "wie": wie, "ident": ident})

    res = run_bass_kernel_spmd(nc, in_maps, list(range(N_CORES)),
                               **({"trace": True} if _trace else {}))
    LAST_RESULTS = res
    outs = res.results

    exc = np.zeros((B, E, T), np.float32)
    for c in range(N_CORES):
        h = outs[c]["hist"].reshape(T, BL, E)             # only first _t_run valid
        exc[c * BL:(c + 1) * BL] = np.transpose(h, (1, 2, 0))
    inh = np.zeros_like(exc)
    inh[:, :, 1:] = exc[:, :, :-1]
    return exc, inh
